# revision 59
# baseline (speedup 1.0000x reference)
"""Trainium2 Bass kernel: multi-head attention (B=2, T=2048, E=1024, H=8, D=512),
bias-free QKV/O projections + RoPE + causal softmax.

Sharding: head-parallel across 8 NeuronCores. Core h computes head h fully:
  qT/kT = RoPE(Wq_h @ x.T), v = x @ Wv_h.T         (projection phase)
  scoresT[k,q] = kT.T @ qT   (per 512-wide q tile; exact-causal: full-width
    [128,512] chunks below the diagonal block; inside the 512x512 diagonal
    block, chunk j covers only its needed q suffix [128j,512) -- widths
    512/384/256/128 -- with one shared [128,128] triangle mask on each
    chunk's first 128 columns; saves ~16k PE cycles/core vs 256-wide tiles)
  probsT = exp(scale*scoresT + mask)               (no max-subtraction: |s|<=9)
  attnT[d,q] = v.T @ probsT ; rowsum = DVE-accumulated exp partials reduced
    across partitions by ONE ones[128,128] matmul per q tile
  out_h = (attnT/rowsum).T @ Wo_h.T                (partial o_proj, [4096,1024])
Host sums the 8 partial outputs (equivalent to the all-reduce after o_proj).

All matmul operands are bf16 (1 cycle/row at any free size, halved DMA/SBUF);
accumulation stays fp32 in PSUM. cos/sin tables are bf16 (halves their
startup DMA; costs ~3e-4 rel err), resident for both batches; x tiles are
double-buffered with the next tile's DMA issued one tile ahead.
x/w inputs arrive host-swizzled into partition-major [128, ...] layout so
each load is one long contiguous DMA row per partition (~1.7x DMA bw).
Boot blob interleaves x-tile0/wv pairs, loaded as four 512KB quarters into
two tiles (>2 DMA writers per tile races in the dep-tracker), so the first
v passes gate on 512KB instead of 1MB. NW warmup matmuls (which also
produce the rowsum `ones` tile and preload the Exp ACT table) keep the PE
busy and the HAM clock ramped while the first loads stream in.

Output DMAs are merged per q-tile (one [512,1024] transfer) except the very
last tile, which runs per-t4/per-half so the final transfer is small. The
last q tile of batch 1 is processed in two 256-column halves (rowsum MM /
reciprocal / normalize / o_proj per half) pipelined against its own diag
chunks and tile-2's o_proj, shrinking the end-of-kernel serial tail.
Unused DMA queue pools are shrunk to 1 (framework default 3x16 queues).

Measured ~364us at 2.4GHz (PE 94% busy, ~348us tensor-active vs ~339us
pure-matmul roofline incl. warmup); remaining overhead is the fixed NEFF
preamble (~7us: engine ifetch + 3 barriers) and epilogue (~7us: ucode
zeroes ~100 semaphores serially on 2 engines) plus ~4us final DMA drain.
"""
from contextlib import ExitStack

import numpy as np

B, T, E, H, D = 2, 2048, 1024, 8, 512
NTOK = B * T
SCALE = float(1.0 / np.sqrt(D))
NEG = -1.0e30
ROPE_BASE = 10000.0
NW = 13                   # warmup matmuls (covers startup DMA + HAM ramp)

PROFILE = False          # set True (e.g. from test.py) to trace core 0
LAST_RESULTS = None      # BassKernelResults of the last run when PROFILE

_CACHE = {}


def _build():
    import concourse.tile as tile
    from concourse import bacc, mybir

    f32 = mybir.dt.float32
    f32r = mybir.dt.float32r
    bf = mybir.dt.bfloat16
    AF = mybir.ActivationFunctionType

    nc = bacc.Bacc("TRN2", target_bir_lowering=False, debug=False,
                   enable_asserts=False, num_devices=8)
    # Shrink the DMA queue pools (default 3 pools x 16 queues): the NEFF
    # epilogue zeroes ~2 semaphores per allocated queue one-by-one on the
    # Tensor/Scalar queues (~115ns each, ~7.7us total at 48 queues). This
    # kernel only issues DMAs from the sync (SP) HWDGE queue and never has
    # more than a handful in flight.
    for q in nc.m.queues:
        if q.name != "qSPDynamicHW":
            q.num_queues = 1
    # x/w arrive host-swizzled into partition-major layout so every DMA row
    # is one long contiguous descriptor (8KB) instead of 1KB fragments —
    # ~1.7x higher effective DMA bandwidth on the loads.
    xsw_d = nc.dram_tensor("xsw", [128, 8, 8, 512], bf,
                           kind="ExternalInput").ap()
    # boot blob, interleaved pairs: slot 2e = x-tile0[e], slot 2e+1 = wv[e].
    # Loaded as four 512KB quarters, so the e0-1 accumulation passes of all
    # four v groups of tile 0 can start after just the first quarter lands.
    boot_d = nc.dram_tensor("boot", [128, 16, 512], bf,
                            kind="ExternalInput").ap()
    wq_d = nc.dram_tensor("wqsw", [128, 8, D], bf, kind="ExternalInput").ap()
    wk_d = nc.dram_tensor("wksw", [128, 8, D], bf, kind="ExternalInput").ap()
    wo_d = nc.dram_tensor("wosw", [128, 4, E], bf, kind="ExternalInput").ap()
    # cos+sin packed in bf16 (halved startup DMA): [2(two), D/2, T]
    cs_d = nc.dram_tensor("csdt", [2, D // 2, T], bf, kind="ExternalInput").ap()
    msk_d = nc.dram_tensor("mtri", [128, 128], f32, kind="ExternalInput").ap()
    out_d = nc.dram_tensor("out", [NTOK, E], bf, kind="ExternalOutput").ap()

    cs_r = cs_d.rearrange("two (fo p) t -> p two fo t", p=128)  # [128,2,2,2048]

    with tile.TileContext(nc) as tc, ExitStack() as top:
        wp = top.enter_context(tc.tile_pool(name="wp", bufs=1))
        wq_t = wp.tile([128, 8, D], bf, tag="wq", name="wq")
        wk_t = wp.tile([128, 8, D], bf, tag="wk", name="wk")
        # boot tiles hold wv (resident all kernel) interleaved with x tile 0;
        # two tiles with two DMA writers each (the tile dep-tracker races
        # with more than two DMA writers on one tile)
        bootA = wp.tile([128, 8, 512], bf, tag="bootA", name="bootA")
        bootB = wp.tile([128, 8, 512], bf, tag="bootB", name="bootB")
        bsl = lambda e: (bootA[:, 2 * (e % 4)] if e < 4
                         else bootB[:, 2 * (e % 4)])
        wv = [(bootA[:, 2 * e + 1] if e < 4 else bootB[:, 2 * (e % 4) + 1])
              for e in range(8)]
        x0sl = [bsl(e) for e in range(8)]
        wo_t = wp.tile([128, 4, E], bf, tag="wo", name="wo")
        wo = [wo_t[:, d] for d in range(4)]
        mtri = wp.tile([128, 128], f32, tag="mtri", name="mtri")
        # bf16 stationary for the rowsum matmuls: an f32r stationary forces
        # the fp32_mode=HIGH 4-byte weight-load path (~475ns/MM vs ~215)
        ones = wp.tile([128, 128], bf, tag="ones", name="ones")
        cst = wp.tile([128, 2, 2, T], bf, tag="cst", name="cst")
        csA = cst[:, 0]
        snA = cst[:, 1]
        xb = [wp.tile([128, 8, 512], bf, tag=f"xb{i}", name=f"xb{i}")
              for i in range(2)]

        # ---- warmup: DMA-independent matmuls lift the PE HAM clock gate
        # while the first loads stream in.
        nc.gpsimd.memset(ones[:], 1.0)
        with ExitStack() as wctx:
            warmp = wctx.enter_context(tc.tile_pool(name="warmp", bufs=1))
            wpp = wctx.enter_context(
                tc.tile_pool(name="wpp", bufs=1, space="PSUM"))
            ones0 = warmp.tile([128, 128], bf, tag="ones0", name="ones0")
            nc.vector.memset(ones0[:], 1.0)
            wsrc = warmp.tile([128, 512], bf, tag="wsrc", name="wsrc")
            nc.vector.memset(wsrc[:], 1.0)
            warm_ps = wpp.tile([128, 512], f32, tag="wps", name="wps")
            for w in range(NW):
                nc.tensor.matmul(warm_ps[:], ones0[:], wsrc[:],
                                 start=(w == 0), stop=(w == NW - 1))
            # touch Exp so its ACT table set loads during the DMA-bound
            # startup instead of at the first score tile
            expre = warmp.tile([128, 1], f32, tag="expre", name="expre")
            nc.scalar.activation(expre[:], warm_ps[:, :1], AF.Exp,
                                 scale=0.001)
            nc.vector.tensor_copy(expre[:], expre[:])

        # ---- initial DMAs, need-ordered on the sync HW queue (the other
        # engine queues proved far slower): first v matmuls need xb0+wv,
        # then wq, cos/sin slice 0 (RoPE of tile 0), wk; everything else
        # (x tile 1, the remaining cos/sin, wo, masks) follows.
        nc.sync.dma_start(bootA[:, 0:4], boot_d[:, 0:4])
        nc.sync.dma_start(bootA[:, 4:8], boot_d[:, 4:8])
        nc.sync.dma_start(bootB[:, 0:4], boot_d[:, 8:12])
        nc.sync.dma_start(bootB[:, 4:8], boot_d[:, 12:16])
        nc.sync.dma_start(wq_t[:], wq_d)
        nc.sync.dma_start(cst[:, :, :, 0:512], cs_r[:, :, :, 0:512])
        nc.sync.dma_start(wk_t[:], wk_d)
        nc.sync.dma_start(xb[1][:], xsw_d[:, 1])
        nc.sync.dma_start(cst[:, :, :, 512:T], cs_r[:, :, :, 512:T])
        nc.sync.dma_start(wo_t[:], wo_d)
        nc.sync.dma_start(mtri[:], msk_d)

        for b in range(B):
            tok0 = b * T
            with ExitStack() as bctx:
                qkv = bctx.enter_context(tc.tile_pool(name="qkv", bufs=1))
                # per-token-tile q/k tiles: scores for q-tile n / k-chunk c
                # then depend only on that tile's RoPE writers, not all four
                qTt = [[qkv.tile([128, 512], bf, tag=f"qT{t}_{d}",
                                 name=f"qT{t}_{d}") for d in range(4)]
                       for t in range(4)]
                kTt = [[qkv.tile([128, 512], bf, tag=f"kT{t}_{d}",
                                 name=f"kT{t}_{d}") for d in range(4)]
                       for t in range(4)]
                vv = [qkv.tile([128, D], bf, tag=f"v{t}", name=f"v{t}")
                      for t in range(16)]

                # ----- projection phase: qT/kT (RoPE'd) and v -----
                with ExitStack() as pctx:
                    tp = pctx.enter_context(tc.tile_pool(name="tp", bufs=4))
                    pp = pctx.enter_context(
                        tc.tile_pool(name="pp", bufs=6, space="PSUM"))
                    ppv = pctx.enter_context(
                        tc.tile_pool(name="ppv", bufs=2, space="PSUM"))

                    for tt in range(4):
                        g = 4 * b + tt
                        s0 = tt * 512
                        # g=0 reads x from the boot blob; later tiles from
                        # the ping-pong buffers (xb[0] first used at g=2)
                        xch = (x0sl if g == 0
                               else [xb[g % 2][:, e] for e in range(8)])
                        # prefetch next x tile (one tile ahead; crosses the
                        # batch boundary so b1's first tile lands during
                        # b0's attention phase). g=0's prefetch (tile 1) was
                        # already issued in the startup batch.
                        if 1 <= g < 7:
                            nc.sync.dma_start(xb[(g + 1) % 2][:],
                                              xsw_d[:, g + 1])

                        def emit_v():
                            for t4 in range(4):
                                ps_t = ppv.tile([128, 512], f32, tag="ppv",
                                                name="ppv")
                                for e in range(8):
                                    nc.tensor.matmul(
                                        ps_t[:],
                                        xch[e][:, t4 * 128:(t4 + 1) * 128],
                                        wv[e][:],
                                        start=(e == 0), stop=(e == 7))
                                nc.scalar.copy(vv[tt * 4 + t4][:], ps_t[:])

                        def emit_v_split():
                            # tile 0 of batch 0: each e-pair's accumulation
                            # passes gate only on its own boot quarter, so
                            # real PE work starts after 512KB of DMA
                            vps = [(ppv if t4 < 2 else pp).tile(
                                [128, 512], f32,
                                tag=("ppv" if t4 < 2 else "pp"),
                                name="vps") for t4 in range(4)]
                            for lo, hi in ((0, 2), (2, 4), (4, 6), (6, 8)):
                                for t4 in range(4):
                                    for e in range(lo, hi):
                                        nc.tensor.matmul(
                                            vps[t4][:],
                                            xch[e][:, t4 * 128:(t4 + 1) * 128],
                                            wv[e][:],
                                            start=(e == 0), stop=(e == 7))
                            for t4 in range(4):
                                nc.scalar.copy(vv[t4][:], vps[t4][:])
                        # v first (its ACT-copy evacuation has no cos/sin
                        # dependency) except on the last token tile, where
                        # qk-first lets the P phase end with a short ACT tail
                        # instead of a long RoPE DVE tail.
                        if tt < 3:
                            if g == 0:
                                emit_v_split()
                            else:
                                emit_v()
                        for w_t, dstT in ((wq_t, qTt[tt]), (wk_t, kTt[tt])):
                            for i, j, fo in ((0, 2, 0), (1, 3, 1)):
                                ps2 = []
                                for dc in (i, j):
                                    ps_t = pp.tile([128, 512], f32, tag="pp",
                                                   name="pp")
                                    for e in range(8):
                                        nc.tensor.matmul(
                                            ps_t[:],
                                            w_t[:, e, dc * 128:(dc + 1) * 128],
                                            xch[e],
                                            start=(e == 0), stop=(e == 7))
                                    ps2.append(ps_t)
                                pi, pj = ps2
                                c_ = csA[:, fo, s0:s0 + 512]
                                s_ = snA[:, fo, s0:s0 + 512]
                                t0 = tp.tile([128, 512], f32, tag="rt", name="rt")
                                t1 = tp.tile([128, 512], f32, tag="rt", name="rt")
                                nc.vector.tensor_mul(t0[:], pi[:], c_)
                                nc.vector.tensor_mul(t1[:], pj[:], s_)
                                nc.vector.tensor_sub(dstT[i][:], t0[:], t1[:])
                                t2 = tp.tile([128, 512], f32, tag="rt", name="rt")
                                t3 = tp.tile([128, 512], f32, tag="rt", name="rt")
                                nc.vector.tensor_mul(t2[:], pi[:], s_)
                                nc.vector.tensor_mul(t3[:], pj[:], c_)
                                nc.vector.tensor_add(dstT[j][:], t2[:], t3[:])
                        if tt == 3:
                            emit_v()

                # ----- attention + o_proj phase -----
                with ExitStack() as actx:
                    ep = actx.enter_context(tc.tile_pool(name="ep", bufs=6))
                    atp = actx.enter_context(tc.tile_pool(name="atp", bufs=1))
                    ivp = actx.enter_context(tc.tile_pool(name="ivp", bufs=2))
                    rsb = actx.enter_context(tc.tile_pool(name="rsb", bufs=2))
                    obp = actx.enter_context(tc.tile_pool(name="obp", bufs=2))
                    scp = actx.enter_context(
                        tc.tile_pool(name="scp", bufs=3, space="PSUM"))
                    app = actx.enter_context(
                        tc.tile_pool(name="app", bufs=1, space="PSUM"))

                    def emit_oproj(n, split=False):
                        # split=True (very last tile): per-t4 output DMAs so
                        # the final transfer is small and starts right after
                        # its ACT-copy instead of waiting for all four.
                        q0 = n * 512
                        ob = obp.tile([128, 4, E], bf, tag="ob", name="ob")
                        for t4 in range(4):
                            for et in range(2):
                                op_ps = scp.tile([128, 512], f32, tag="sc",
                                                 name="sc")
                                for dc in range(4):
                                    nc.tensor.matmul(
                                        op_ps[:],
                                        at_sb[n % 2][dc][:, t4 * 128:(t4 + 1) * 128],
                                        wo[dc][:, et * 512:(et + 1) * 512],
                                        start=(dc == 0), stop=(dc == 3))
                                nc.scalar.copy(
                                    ob[:, t4, et * 512:(et + 1) * 512], op_ps[:])
                            if split:
                                r0 = tok0 + q0 + t4 * 128
                                nc.sync.dma_start(out_d[r0:r0 + 128, :],
                                                  ob[:, t4])
                        if not split:
                            r0 = tok0 + q0
                            nc.sync.dma_start(
                                out_d[r0:r0 + 512, :].rearrange(
                                    "(f p) e -> p f e", p=128),
                                ob[:])

                    at_sb = {0: None, 1: None}
                    for n in range(4):
                        q0 = n * 512
                        noff = 4 * n          # full-width k chunks below diag
                        attn_ps = [app.tile([128, 512], f32, tag=f"attn{d}",
                                            name=f"attn{d}") for d in range(4)]

                        rs_acc = rsb.tile([128, 512], bf, tag="rsa",
                                          name="rsa")

                        # rowsum partial accumulation (DVE); the first two
                        # full-width exp tiles are paired into one 2-input
                        # add; diag chunks add into suffix ranges [qoff:512]
                        rst = {"f": None, "fi": False}

                        def rs_add(ex, qoff, w, rst=rst, rs_acc=rs_acc):
                            if not rst["fi"]:
                                if w == 512:
                                    if rst["f"] is None:
                                        rst["f"] = ex
                                        return
                                    nc.vector.tensor_add(
                                        rs_acc[:], rst["f"][:], ex[:])
                                    rst["fi"] = True
                                    return
                                # lone 512-wide first touch (n=0 diag j=0):
                                # materialize it, then add this suffix
                                nc.vector.tensor_copy(rs_acc[:], rst["f"][:])
                                rst["fi"] = True
                            dst = rs_acc[:, qoff:qoff + w]
                            nc.vector.tensor_add(dst, dst, ex[:])

                        def emit_pv(ex, c, qoff, w, st, sp,
                                    attn_ps=attn_ps):
                            for dc in range(4):
                                nc.tensor.matmul(
                                    attn_ps[dc][:, qoff:qoff + w],
                                    vv[c][:, dc * 128:(dc + 1) * 128], ex[:],
                                    start=st, stop=sp)

                        pending = []

                        def push(entry):
                            # rowsum add at push time (it only needs the exp
                            # tile, ready now) — keeps the DVE chain tracking
                            # the exp stream so the q-tile's trailing
                            # rs->recip->normalize path isn't serialized
                            # behind a flush backlog
                            rs_add(entry[0], entry[2], entry[3])
                            pending.append(entry)
                            if len(pending) > 4:
                                emit_pv(*pending.pop(0))

                        # full-width chunks strictly below the diagonal block
                        for c in range(noff):
                            sc_ps = scp.tile([128, 512], f32, tag="sc",
                                             name="sc")
                            for dc in range(4):
                                nc.tensor.matmul(
                                    sc_ps[:],
                                    kTt[c // 4][dc][:, (c % 4) * 128:
                                                    (c % 4 + 1) * 128],
                                    qTt[n][dc][:],
                                    start=(dc == 0), stop=(dc == 3))
                            ex = ep.tile([128, 512], bf, tag="ex", name="ex")
                            nc.scalar.activation(ex[:], sc_ps[:], AF.Exp,
                                                 scale=SCALE)
                            push((ex, c, 0, 512, c == 0, False))
                        # diagonal 512x512 block: chunk j (k in [128j,128j+128)
                        # within the block) covers only its causally-needed q
                        # suffix [128j, 512); the first 128 q columns of each
                        # chunk sit on the diagonal and get the triangle mask
                        def diag_chunk(j, immediate=False):
                            c = noff + j
                            qoff = j * 128
                            w = 512 - qoff
                            sc_ps = scp.tile([128, 512], f32, tag="sc",
                                             name="sc")
                            for dc in range(4):
                                nc.tensor.matmul(
                                    sc_ps[:, 0:w],
                                    kTt[c // 4][dc][:, (c % 4) * 128:
                                                    (c % 4 + 1) * 128],
                                    qTt[n][dc][:, qoff:512],
                                    start=(dc == 0), stop=(dc == 3))
                            nc.vector.tensor_add(
                                sc_ps[:, 0:128], sc_ps[:, 0:128], mtri[:])
                            ex = ep.tile([128, w], bf,
                                         tag=("ex" if j == 0 else f"exd{j}"),
                                         name="exd")
                            nc.scalar.activation(ex[:], sc_ps[:, 0:w],
                                                 AF.Exp, scale=SCALE)
                            entry = (ex, c, qoff, w,
                                     noff == 0 and j == 0, j == 3)
                            if immediate:
                                rs_add(ex, qoff, w)
                                return entry
                            push(entry)

                        def normalize(dst, lo, w, inv):
                            for dc in range(4):
                                nc.vector.tensor_mul(dst[dc][:, lo:lo + w],
                                                     attn_ps[dc][:, lo:lo + w],
                                                     inv[:])

                        last = (b == B - 1 and n == 3)
                        if not last:
                            for j in range(4):
                                diag_chunk(j)
                            for entry in pending:
                                emit_pv(*entry)
                            # cross-partition reduce of the exp partials: ONE
                            # ones-matmul per q tile (broadcasts rowsum to all
                            # partitions), then normalize + evacuate
                            rs_ps = app.tile([128, 512], f32, tag="rs",
                                             name="rs")
                            nc.tensor.matmul(rs_ps[:], ones[:], rs_acc[:],
                                             start=True, stop=True)
                            inv = ivp.tile([128, 512], f32, tag="inv",
                                           name="inv")
                            nc.vector.reciprocal(inv[:], rs_ps[:])
                            at_sb[n % 2] = [
                                atp.tile([128, 512], bf, tag=f"at{n % 2}_{dc}",
                                         name=f"at{n % 2}_{dc}")
                                for dc in range(4)]
                            normalize(at_sb[n % 2], 0, 512, inv)
                            if n > 0:
                                emit_oproj(n - 1)
                        else:
                            # last q tile of the last batch: process in two
                            # 256-wide halves so rowsum/normalize/o_proj of
                            # the left half overlap the right half's PV, and
                            # the end-of-kernel serial tail shrinks
                            diag_chunk(0)
                            diag_chunk(1)
                            for entry in pending:
                                emit_pv(*entry)
                            at_sb[1] = [
                                atp.tile([128, 512], bf, tag=f"at1_{dc}",
                                         name=f"at1_{dc}")
                                for dc in range(4)]
                            ob = obp.tile([128, 4, E], bf, tag="ob",
                                          name="ob")

                            def half(h):
                                lo = h * 256
                                nc.tensor.matmul(rs_ps[:, lo:lo + 256],
                                                 ones[:],
                                                 rs_acc[:, lo:lo + 256],
                                                 start=True, stop=True)
                                inv = ivp.tile([128, 256], f32,
                                               tag=f"invh{h}", name="invh")
                                nc.vector.reciprocal(inv[:],
                                                     rs_ps[:, lo:lo + 256])
                                normalize(at_sb[1], lo, 256, inv)

                            def oproj_t4(t4):
                                r0 = tok0 + q0 + t4 * 128
                                for et in range(2):
                                    op_ps = scp.tile([128, 512], f32,
                                                     tag="sc", name="sc")
                                    for dc in range(4):
                                        nc.tensor.matmul(
                                            op_ps[:],
                                            at_sb[1][dc][:, t4 * 128:
                                                         (t4 + 1) * 128],
                                            wo[dc][:, et * 512:(et + 1) * 512],
                                            start=(dc == 0), stop=(dc == 3))
                                    nc.scalar.copy(
                                        ob[:, t4, et * 512:(et + 1) * 512],
                                        op_ps[:])
                                    if t4 == 3:
                                        # very last tile: per-half DMAs so the
                                        # final transfer is 128KB and starts
                                        # one ACT-copy earlier
                                        nc.sync.dma_start(
                                            out_d[r0:r0 + 128,
                                                  et * 512:(et + 1) * 512],
                                            ob[:, t4, et * 512:(et + 1) * 512])
                                if t4 < 3:
                                    nc.sync.dma_start(out_d[r0:r0 + 128, :],
                                                      ob[:, t4])

                            # tile 2's o_proj runs as per-t4 filler: between
                            # each diag chunk's scores and its PV (hiding the
                            # mask->exp latency), then overlapping the two
                            # halves' normalize chains
                            ob2 = obp.tile([128, 4, E], bf, tag="ob",
                                           name="ob2")

                            def op2_t4(t4):
                                for et in range(2):
                                    op_ps = scp.tile([128, 512], f32,
                                                     tag="sc", name="sc")
                                    for dc in range(4):
                                        nc.tensor.matmul(
                                            op_ps[:],
                                            at_sb[0][dc][:, t4 * 128:
                                                         (t4 + 1) * 128],
                                            wo[dc][:, et * 512:(et + 1) * 512],
                                            start=(dc == 0), stop=(dc == 3))
                                    nc.scalar.copy(
                                        ob2[:, t4, et * 512:(et + 1) * 512],
                                        op_ps[:])

                            e2 = diag_chunk(2, immediate=True)
                            op2_t4(0)
                            emit_pv(*e2)
                            e3 = diag_chunk(3, immediate=True)
                            op2_t4(1)
                            emit_pv(*e3)
                            rs_ps = app.tile([128, 512], f32, tag="rs",
                                             name="rs")
                            half(0)
                            half(1)
                            op2_t4(2)
                            op2_t4(3)
                            r2 = tok0 + 2 * 512
                            nc.sync.dma_start(
                                out_d[r2:r2 + 512, :].rearrange(
                                    "(f p) e -> p f e", p=128),
                                ob2[:])
                            for t4 in range(4):
                                oproj_t4(t4)
                    if not (b == B - 1):
                        emit_oproj(3)
    nc.compile()
    return nc


def _host_tables():
    inv_freq = 1.0 / (ROPE_BASE ** (np.arange(0, D, 2, dtype=np.float64) / D))
    ang = np.arange(T, dtype=np.float64)[:, None] * inv_freq[None, :]  # [T, D/2]
    import ml_dtypes
    cosdt = np.cos(ang).T.astype(ml_dtypes.bfloat16)                   # [D/2, T]
    sindt = np.sin(ang).T.astype(ml_dtypes.bfloat16)
    csdt = np.ascontiguousarray(np.stack([cosdt, sindt]))  # [2, D/2, T]
    kk = np.arange(128)[:, None]
    qq = np.arange(128)[None, :]
    mtri = np.where(kk <= qq, 0.0, NEG).astype(np.float32)  # [128(k),128(q)]
    return csdt, mtri


def kernel(x, Wq, Wk, Wv, Wo):
    global LAST_RESULTS
    import ml_dtypes
    from concourse import bass_utils

    if "nc" not in _CACHE:
        _CACHE["nc"] = _build()
    nc = _CACHE["nc"]

    bf = ml_dtypes.bfloat16
    x = np.asarray(x, dtype=np.float32)
    Wq = np.asarray(Wq, dtype=np.float32)
    Wk = np.asarray(Wk, dtype=np.float32)
    Wv = np.asarray(Wv, dtype=np.float32)
    Wo = np.asarray(Wo, dtype=np.float32)

    # partition-major swizzles (see kernel docstring): long contiguous DMA rows
    xT = x.reshape(NTOK, E).T.astype(bf)                          # [E, NTOK]
    xsw = np.ascontiguousarray(
        xT.reshape(8, 128, 8, 512).transpose(1, 2, 0, 3))  # [128, g, eo, t]

    def wsw(W, h):  # [E, D] head slice -> [128, eo, d]
        wT = W[h * D:(h + 1) * D, :].T.astype(bf)
        return np.ascontiguousarray(wT.reshape(8, 128, D).transpose(1, 0, 2))

    csdt, mtri = _host_tables()

    in_maps = []
    for h in range(H):
        woT = Wo[:, h * D:(h + 1) * D].T.astype(bf)               # [D, E]
        wvs = wsw(Wv, h)
        boot = np.empty((128, 16, 512), dtype=bf)                 # interleaved
        boot[:, 0::2] = xsw[:, 0]
        boot[:, 1::2] = wvs
        in_maps.append({
            "xsw": xsw,
            "boot": boot,
            "wqsw": wsw(Wq, h),
            "wksw": wsw(Wk, h),
            "wosw": np.ascontiguousarray(
                woT.reshape(4, 128, E).transpose(1, 0, 2)),
            "csdt": csdt,
            "mtri": mtri,
        })

    kwargs = {}
    if PROFILE:
        import sys
        import types
        import trn_agent_boot.trn_boot as _tb
        hook = _tb._ntff_profile_via_ctypes("/opt/axon/libaxon_pjrt.so")
        mod = types.ModuleType("antenv.axon_hooks")
        mod.get_axon_ntff_profile_hook = lambda: hook
        mod.set_axon_ntff_profile_hook = lambda h_: None
        sys.modules["antenv.axon_hooks"] = mod
        bass_utils.upload_artifacts = lambda tmpdir: tmpdir
        kwargs = dict(trace=True, trace_cores=[0])

    res = bass_utils.run_bass_kernel_spmd(
        nc, in_maps, core_ids=list(range(H)), **kwargs)
    LAST_RESULTS = res

    out = res.results[0]["out"].astype(np.float32)
    for h in range(1, H):
        out += res.results[h]["out"].astype(np.float32)
    return out.reshape(B, T, E)



# revision 61
# speedup vs baseline: 1.0079x; 1.0079x over previous
"""Trainium2 Bass kernel: multi-head attention (B=2, T=2048, E=1024, H=8, D=512),
bias-free QKV/O projections + RoPE + causal softmax.

Sharding: head-parallel across 8 NeuronCores. Core h computes head h fully:
  qT/kT = RoPE(Wq_h @ x.T), v = x @ Wv_h.T         (projection phase)
  scoresT[k,q] = kT.T @ qT   (per 512-wide q tile; exact-causal: full-width
    [128,512] chunks below the diagonal block; inside the 512x512 diagonal
    block, chunk j covers only its needed q suffix [128j,512) -- widths
    512/384/256/128 -- with one shared [128,128] triangle mask on each
    chunk's first 128 columns; saves ~16k PE cycles/core vs 256-wide tiles)
  probsT = exp(scale*scoresT + mask)               (no max-subtraction: |s|<=9)
  attnT[d,q] = v.T @ probsT ; rowsum = DVE-accumulated exp partials reduced
    across partitions by ONE ones[128,128] matmul per q tile
  out_h = (attnT/rowsum).T @ Wo_h.T                (partial o_proj, [4096,1024])
Host sums the 8 partial outputs (equivalent to the all-reduce after o_proj).

All matmul operands are bf16 (1 cycle/row at any free size, halved DMA/SBUF);
accumulation stays fp32 in PSUM. cos/sin tables are bf16 (halves their
startup DMA; costs ~3e-4 rel err), resident for both batches; x tiles are
double-buffered with the next tile's DMA issued one tile ahead.
x/w inputs arrive host-swizzled into partition-major [128, ...] layout so
each load is one long contiguous DMA row per partition (~1.7x DMA bw).
Boot blob interleaves x-tile0/wv pairs, loaded as four 512KB quarters into
two tiles (>2 DMA writers per tile races in the dep-tracker), so the first
v passes gate on 512KB instead of 1MB. NW warmup matmuls (which also
produce the rowsum `ones` tile and preload the Exp ACT table) keep the PE
busy and the HAM clock ramped while the first loads stream in.

Output DMAs are merged per q-tile (one [512,1024] transfer) except the very
last tile, which runs per-t4/per-half so the final transfer is small. The
last q tile of batch 1 is processed in two 256-column halves (rowsum MM /
reciprocal / normalize / o_proj per half) pipelined against its own diag
chunks and tile-2's o_proj, shrinking the end-of-kernel serial tail.
Unused DMA queue pools are shrunk to 1 (framework default 3x16 queues).

Measured ~364us at 2.4GHz (PE 94% busy, ~348us tensor-active vs ~339us
pure-matmul roofline incl. warmup); remaining overhead is the fixed NEFF
preamble (~7us: engine ifetch + 3 barriers) and epilogue (~7us: ucode
zeroes ~100 semaphores serially on 2 engines) plus ~4us final DMA drain.
"""
from contextlib import ExitStack

import numpy as np

B, T, E, H, D = 2, 2048, 1024, 8, 512
NTOK = B * T
SCALE = float(1.0 / np.sqrt(D))
NEG = -1.0e30
ROPE_BASE = 10000.0
NW = 13                   # warmup matmuls (covers startup DMA + HAM ramp)

PROFILE = False          # set True (e.g. from test.py) to trace core 0
LAST_RESULTS = None      # BassKernelResults of the last run when PROFILE

_CACHE = {}


def _build():
    import concourse.tile as tile
    from concourse import bacc, mybir

    f32 = mybir.dt.float32
    f32r = mybir.dt.float32r
    bf = mybir.dt.bfloat16
    AF = mybir.ActivationFunctionType

    nc = bacc.Bacc("TRN2", target_bir_lowering=False, debug=False,
                   enable_asserts=False, num_devices=8)
    # Shrink the DMA queue pools (default 3 pools x 16 queues): the NEFF
    # epilogue zeroes ~2 semaphores per allocated queue one-by-one on the
    # Tensor/Scalar queues (~115ns each, ~7.7us total at 48 queues). This
    # kernel only issues DMAs from the sync (SP) HWDGE queue and never has
    # more than a handful in flight.
    for q in nc.m.queues:
        if q.name != "qSPDynamicHW":
            q.num_queues = 1
    # x/w arrive host-swizzled into partition-major layout so every DMA row
    # is one long contiguous descriptor (8KB) instead of 1KB fragments —
    # ~1.7x higher effective DMA bandwidth on the loads.
    xsw_d = nc.dram_tensor("xsw", [128, 8, 8, 512], bf,
                           kind="ExternalInput").ap()
    # boot blob, interleaved pairs: slot 2e = x-tile0[e], slot 2e+1 = wv[e].
    # Loaded as four 512KB quarters, so the e0-1 accumulation passes of all
    # four v groups of tile 0 can start after just the first quarter lands.
    boot_d = nc.dram_tensor("boot", [128, 16, 512], bf,
                            kind="ExternalInput").ap()
    wq_d = nc.dram_tensor("wqsw", [128, 8, D], bf, kind="ExternalInput").ap()
    wk_d = nc.dram_tensor("wksw", [128, 8, D], bf, kind="ExternalInput").ap()
    wo_d = nc.dram_tensor("wosw", [128, 4, E], bf, kind="ExternalInput").ap()
    # cos+sin packed in bf16 (halved startup DMA): [2(two), D/2, T]
    cs_d = nc.dram_tensor("csdt", [2, D // 2, T], bf, kind="ExternalInput").ap()
    msk_d = nc.dram_tensor("mtri", [128, 128], f32, kind="ExternalInput").ap()
    out_d = nc.dram_tensor("out", [NTOK, E], bf, kind="ExternalOutput").ap()

    cs_r = cs_d.rearrange("two (fo p) t -> p two fo t", p=128)  # [128,2,2,2048]

    with tile.TileContext(nc) as tc, ExitStack() as top:
        wp = top.enter_context(tc.tile_pool(name="wp", bufs=1))
        wq_t = wp.tile([128, 8, D], bf, tag="wq", name="wq")
        wk_t = wp.tile([128, 8, D], bf, tag="wk", name="wk")
        # boot tiles hold wv (resident all kernel) interleaved with x tile 0;
        # two tiles with two DMA writers each (the tile dep-tracker races
        # with more than two DMA writers on one tile)
        bootA = wp.tile([128, 8, 512], bf, tag="bootA", name="bootA")
        bootB = wp.tile([128, 8, 512], bf, tag="bootB", name="bootB")
        bsl = lambda e: (bootA[:, 2 * (e % 4)] if e < 4
                         else bootB[:, 2 * (e % 4)])
        wv = [(bootA[:, 2 * e + 1] if e < 4 else bootB[:, 2 * (e % 4) + 1])
              for e in range(8)]
        x0sl = [bsl(e) for e in range(8)]
        wo_t = wp.tile([128, 4, E], bf, tag="wo", name="wo")
        wo = [wo_t[:, d] for d in range(4)]
        mtri = wp.tile([128, 128], f32, tag="mtri", name="mtri")
        # bf16 stationary for the rowsum matmuls: an f32r stationary forces
        # the fp32_mode=HIGH 4-byte weight-load path (~475ns/MM vs ~215)
        ones = wp.tile([128, 128], bf, tag="ones", name="ones")
        cst = wp.tile([128, 2, 2, T], bf, tag="cst", name="cst")
        csA = cst[:, 0]
        snA = cst[:, 1]
        xb = [wp.tile([128, 8, 512], bf, tag=f"xb{i}", name=f"xb{i}")
              for i in range(2)]

        # ---- warmup: DMA-independent matmuls lift the PE HAM clock gate
        # while the first loads stream in.
        nc.gpsimd.memset(ones[:], 1.0)
        with ExitStack() as wctx:
            warmp = wctx.enter_context(tc.tile_pool(name="warmp", bufs=1))
            wpp = wctx.enter_context(
                tc.tile_pool(name="wpp", bufs=1, space="PSUM"))
            ones0 = warmp.tile([128, 128], bf, tag="ones0", name="ones0")
            nc.vector.memset(ones0[:], 1.0)
            wsrc = warmp.tile([128, 512], bf, tag="wsrc", name="wsrc")
            nc.vector.memset(wsrc[:], 1.0)
            warm_ps = wpp.tile([128, 512], f32, tag="wps", name="wps")
            for w in range(NW):
                nc.tensor.matmul(warm_ps[:], ones0[:], wsrc[:],
                                 start=(w == 0), stop=(w == NW - 1))
            # touch Exp so its ACT table set loads during the DMA-bound
            # startup instead of at the first score tile
            expre = warmp.tile([128, 1], f32, tag="expre", name="expre")
            nc.scalar.activation(expre[:], warm_ps[:, :1], AF.Exp,
                                 scale=0.001)
            nc.vector.tensor_copy(expre[:], expre[:])

        # ---- initial DMAs, need-ordered on the sync HW queue (the other
        # engine queues proved far slower): first v matmuls need xb0+wv,
        # then wq, cos/sin slice 0 (RoPE of tile 0), wk; everything else
        # (x tile 1, the remaining cos/sin, wo, masks) follows.
        nc.sync.dma_start(bootA[:, 0:4], boot_d[:, 0:4])
        nc.sync.dma_start(bootA[:, 4:8], boot_d[:, 4:8])
        nc.sync.dma_start(bootB[:, 0:4], boot_d[:, 8:12])
        nc.sync.dma_start(bootB[:, 4:8], boot_d[:, 12:16])
        nc.sync.dma_start(wq_t[:], wq_d)
        nc.sync.dma_start(cst[:, :, :, 0:512], cs_r[:, :, :, 0:512])
        nc.sync.dma_start(wk_t[:], wk_d)
        nc.sync.dma_start(xb[1][:], xsw_d[:, 1])
        nc.sync.dma_start(cst[:, :, :, 512:T], cs_r[:, :, :, 512:T])
        nc.sync.dma_start(wo_t[:], wo_d)
        nc.sync.dma_start(mtri[:], msk_d)

        for b in range(B):
            tok0 = b * T
            with ExitStack() as bctx:
                qkv = bctx.enter_context(tc.tile_pool(name="qkv", bufs=1))
                # per-token-tile q/k tiles: scores for q-tile n / k-chunk c
                # then depend only on that tile's RoPE writers, not all four
                qTt = [[qkv.tile([128, 512], bf, tag=f"qT{t}_{d}",
                                 name=f"qT{t}_{d}") for d in range(4)]
                       for t in range(4)]
                kTt = [[qkv.tile([128, 512], bf, tag=f"kT{t}_{d}",
                                 name=f"kT{t}_{d}") for d in range(4)]
                       for t in range(4)]
                vv = [qkv.tile([128, D], bf, tag=f"v{t}", name=f"v{t}")
                      for t in range(16)]

                # ----- projection phase: qT/kT (RoPE'd) and v -----
                with ExitStack() as pctx:
                    tp = pctx.enter_context(tc.tile_pool(name="tp", bufs=4))
                    pp = pctx.enter_context(
                        tc.tile_pool(name="pp", bufs=6, space="PSUM"))
                    ppv = pctx.enter_context(
                        tc.tile_pool(name="ppv", bufs=2, space="PSUM"))

                    for tt in range(4):
                        g = 4 * b + tt
                        s0 = tt * 512
                        # g=0 reads x from the boot blob; later tiles from
                        # the ping-pong buffers (xb[0] first used at g=2)
                        xch = (x0sl if g == 0
                               else [xb[g % 2][:, e] for e in range(8)])
                        # prefetch next x tile (one tile ahead; crosses the
                        # batch boundary so b1's first tile lands during
                        # b0's attention phase). g=0's prefetch (tile 1) was
                        # already issued in the startup batch.
                        if 1 <= g < 7:
                            nc.sync.dma_start(xb[(g + 1) % 2][:],
                                              xsw_d[:, g + 1])

                        def emit_v():
                            for t4 in range(4):
                                ps_t = ppv.tile([128, 512], f32, tag="ppv",
                                                name="ppv")
                                for e in range(8):
                                    nc.tensor.matmul(
                                        ps_t[:],
                                        xch[e][:, t4 * 128:(t4 + 1) * 128],
                                        wv[e][:],
                                        start=(e == 0), stop=(e == 7))
                                nc.scalar.copy(vv[tt * 4 + t4][:], ps_t[:])

                        def emit_v_split():
                            # tile 0 of batch 0: each e-pair's accumulation
                            # passes gate only on its own boot quarter, so
                            # real PE work starts after 512KB of DMA
                            vps = [(ppv if t4 < 2 else pp).tile(
                                [128, 512], f32,
                                tag=("ppv" if t4 < 2 else "pp"),
                                name="vps") for t4 in range(4)]
                            for lo, hi in ((0, 2), (2, 4), (4, 6), (6, 8)):
                                for t4 in range(4):
                                    for e in range(lo, hi):
                                        nc.tensor.matmul(
                                            vps[t4][:],
                                            xch[e][:, t4 * 128:(t4 + 1) * 128],
                                            wv[e][:],
                                            start=(e == 0), stop=(e == 7))
                            for t4 in range(4):
                                nc.scalar.copy(vv[t4][:], vps[t4][:])
                        # v first (its ACT-copy evacuation has no cos/sin
                        # dependency) except on the last token tile, where
                        # qk-first lets the P phase end with a short ACT tail
                        # instead of a long RoPE DVE tail.
                        if tt < 3:
                            if g == 0:
                                emit_v_split()
                            else:
                                emit_v()
                        for w_t, dstT in ((wq_t, qTt[tt]), (wk_t, kTt[tt])):
                            for i, j, fo in ((0, 2, 0), (1, 3, 1)):
                                ps2 = []
                                for dc in (i, j):
                                    ps_t = pp.tile([128, 512], f32, tag="pp",
                                                   name="pp")
                                    for e in range(8):
                                        nc.tensor.matmul(
                                            ps_t[:],
                                            w_t[:, e, dc * 128:(dc + 1) * 128],
                                            xch[e],
                                            start=(e == 0), stop=(e == 7))
                                    ps2.append(ps_t)
                                pi, pj = ps2
                                c_ = csA[:, fo, s0:s0 + 512]
                                s_ = snA[:, fo, s0:s0 + 512]
                                t0 = tp.tile([128, 512], f32, tag="rt", name="rt")
                                t1 = tp.tile([128, 512], f32, tag="rt", name="rt")
                                nc.vector.tensor_mul(t0[:], pi[:], c_)
                                nc.vector.tensor_mul(t1[:], pj[:], s_)
                                nc.vector.tensor_sub(dstT[i][:], t0[:], t1[:])
                                t2 = tp.tile([128, 512], f32, tag="rt", name="rt")
                                t3 = tp.tile([128, 512], f32, tag="rt", name="rt")
                                nc.vector.tensor_mul(t2[:], pi[:], s_)
                                nc.vector.tensor_mul(t3[:], pj[:], c_)
                                nc.vector.tensor_add(dstT[j][:], t2[:], t3[:])
                        if tt == 3:
                            emit_v()

                # ----- attention + o_proj phase -----
                with ExitStack() as actx:
                    ep = actx.enter_context(tc.tile_pool(name="ep", bufs=6))
                    atp = actx.enter_context(tc.tile_pool(name="atp", bufs=1))
                    ivp = actx.enter_context(tc.tile_pool(name="ivp", bufs=2))
                    rsb = actx.enter_context(tc.tile_pool(name="rsb", bufs=2))
                    obp = actx.enter_context(tc.tile_pool(name="obp", bufs=2))
                    scp = actx.enter_context(
                        tc.tile_pool(name="scp", bufs=3, space="PSUM"))
                    app = actx.enter_context(
                        tc.tile_pool(name="app", bufs=1, space="PSUM"))

                    def emit_oproj(n, split=False):
                        # split=True (very last tile): per-t4 output DMAs so
                        # the final transfer is small and starts right after
                        # its ACT-copy instead of waiting for all four.
                        q0 = n * 512
                        ob = obp.tile([128, 4, E], bf, tag="ob", name="ob")
                        for t4 in range(4):
                            for et in range(2):
                                op_ps = scp.tile([128, 512], f32, tag="sc",
                                                 name="sc")
                                for dc in range(4):
                                    nc.tensor.matmul(
                                        op_ps[:],
                                        at_sb[n % 2][dc][:, t4 * 128:(t4 + 1) * 128],
                                        wo[dc][:, et * 512:(et + 1) * 512],
                                        start=(dc == 0), stop=(dc == 3))
                                nc.scalar.copy(
                                    ob[:, t4, et * 512:(et + 1) * 512], op_ps[:])
                            if split:
                                r0 = tok0 + q0 + t4 * 128
                                nc.sync.dma_start(out_d[r0:r0 + 128, :],
                                                  ob[:, t4])
                        if not split:
                            r0 = tok0 + q0
                            nc.sync.dma_start(
                                out_d[r0:r0 + 512, :].rearrange(
                                    "(f p) e -> p f e", p=128),
                                ob[:])

                    at_sb = {0: None, 1: None}
                    for n in range(4):
                        q0 = n * 512
                        noff = 4 * n          # full-width k chunks below diag
                        attn_ps = [app.tile([128, 512], f32, tag=f"attn{d}",
                                            name=f"attn{d}") for d in range(4)]

                        rs_acc = rsb.tile([128, 512], bf, tag="rsa",
                                          name="rsa")

                        # rowsum partial accumulation (DVE); the first two
                        # full-width exp tiles are paired into one 2-input
                        # add; diag chunks add into suffix ranges [qoff:512]
                        rst = {"f": None, "fi": False}

                        def rs_add(ex, qoff, w, rst=rst, rs_acc=rs_acc):
                            if not rst["fi"]:
                                if w == 512:
                                    if rst["f"] is None:
                                        rst["f"] = ex
                                        return
                                    nc.vector.tensor_add(
                                        rs_acc[:], rst["f"][:], ex[:])
                                    rst["fi"] = True
                                    return
                                # lone 512-wide first touch (n=0 diag j=0):
                                # materialize it, then add this suffix
                                nc.vector.tensor_copy(rs_acc[:], rst["f"][:])
                                rst["fi"] = True
                            dst = rs_acc[:, qoff:qoff + w]
                            nc.vector.tensor_add(dst, dst, ex[:])

                        def emit_pv(ex, c, qoff, w, st, sp,
                                    attn_ps=attn_ps):
                            for dc in range(4):
                                nc.tensor.matmul(
                                    attn_ps[dc][:, qoff:qoff + w],
                                    vv[c][:, dc * 128:(dc + 1) * 128], ex[:],
                                    start=st, stop=sp)

                        pending = []

                        def push(entry):
                            # rowsum add at push time (it only needs the exp
                            # tile, ready now) — keeps the DVE chain tracking
                            # the exp stream so the q-tile's trailing
                            # rs->recip->normalize path isn't serialized
                            # behind a flush backlog
                            rs_add(entry[0], entry[2], entry[3])
                            pending.append(entry)
                            if len(pending) > 4:
                                emit_pv(*pending.pop(0))

                        # full-width chunks strictly below the diagonal block
                        for c in range(noff):
                            sc_ps = scp.tile([128, 512], f32, tag="sc",
                                             name="sc")
                            for dc in range(4):
                                nc.tensor.matmul(
                                    sc_ps[:],
                                    kTt[c // 4][dc][:, (c % 4) * 128:
                                                    (c % 4 + 1) * 128],
                                    qTt[n][dc][:],
                                    start=(dc == 0), stop=(dc == 3))
                            ex = ep.tile([128, 512], bf, tag="ex", name="ex")
                            nc.scalar.activation(ex[:], sc_ps[:], AF.Exp,
                                                 scale=SCALE)
                            push((ex, c, 0, 512, c == 0, False))
                        # diagonal 512x512 block: chunk j (k in [128j,128j+128)
                        # within the block) covers only its causally-needed q
                        # suffix [128j, 512); the first 128 q columns of each
                        # chunk sit on the diagonal and get the triangle mask
                        def diag_chunk(j, immediate=False):
                            c = noff + j
                            qoff = j * 128
                            w = 512 - qoff
                            sc_ps = scp.tile([128, 512], f32, tag="sc",
                                             name="sc")
                            for dc in range(4):
                                nc.tensor.matmul(
                                    sc_ps[:, 0:w],
                                    kTt[c // 4][dc][:, (c % 4) * 128:
                                                    (c % 4 + 1) * 128],
                                    qTt[n][dc][:, qoff:512],
                                    start=(dc == 0), stop=(dc == 3))
                            nc.vector.tensor_add(
                                sc_ps[:, 0:128], sc_ps[:, 0:128], mtri[:])
                            ex = ep.tile([128, w], bf,
                                         tag=("ex" if j == 0 else f"exd{j}"),
                                         name="exd")
                            nc.scalar.activation(ex[:], sc_ps[:, 0:w],
                                                 AF.Exp, scale=SCALE)
                            entry = (ex, c, qoff, w,
                                     noff == 0 and j == 0, j == 3)
                            if immediate:
                                rs_add(ex, qoff, w)
                                return entry
                            push(entry)

                        def normalize(dst, lo, w, inv):
                            for dc in range(4):
                                nc.vector.tensor_mul(dst[dc][:, lo:lo + w],
                                                     attn_ps[dc][:, lo:lo + w],
                                                     inv[:])

                        last = (b == B - 1 and n == 3)
                        if not last:
                            for j in range(4):
                                diag_chunk(j)
                            for entry in pending:
                                emit_pv(*entry)
                            # cross-partition reduce of the exp partials: ONE
                            # ones-matmul per q tile (broadcasts rowsum to all
                            # partitions), then normalize + evacuate
                            rs_ps = app.tile([128, 512], f32, tag="rs",
                                             name="rs")
                            nc.tensor.matmul(rs_ps[:], ones[:], rs_acc[:],
                                             start=True, stop=True)
                            inv = ivp.tile([128, 512], f32, tag="inv",
                                           name="inv")
                            nc.vector.reciprocal_approx_fast(
                                out=inv[:], in_=rs_ps[:])
                            at_sb[n % 2] = [
                                atp.tile([128, 512], bf, tag=f"at{n % 2}_{dc}",
                                         name=f"at{n % 2}_{dc}")
                                for dc in range(4)]
                            normalize(at_sb[n % 2], 0, 512, inv)
                            if n > 0:
                                emit_oproj(n - 1)
                        else:
                            # last q tile of the last batch: process in two
                            # 256-wide halves so rowsum/normalize/o_proj of
                            # the left half overlap the right half's PV, and
                            # the end-of-kernel serial tail shrinks
                            diag_chunk(0)
                            diag_chunk(1)
                            for entry in pending:
                                emit_pv(*entry)
                            at_sb[1] = [
                                atp.tile([128, 512], bf, tag=f"at1_{dc}",
                                         name=f"at1_{dc}")
                                for dc in range(4)]
                            ob = obp.tile([128, 4, E], bf, tag="ob",
                                          name="ob")

                            def half(h):
                                lo = h * 256
                                nc.tensor.matmul(rs_ps[:, lo:lo + 256],
                                                 ones[:],
                                                 rs_acc[:, lo:lo + 256],
                                                 start=True, stop=True)
                                inv = ivp.tile([128, 256], f32,
                                               tag=f"invh{h}", name="invh")
                                nc.vector.reciprocal_approx_fast(
                                    out=inv[:], in_=rs_ps[:, lo:lo + 256])
                                normalize(at_sb[1], lo, 256, inv)

                            def oproj_t4(t4):
                                r0 = tok0 + q0 + t4 * 128
                                for et in range(2):
                                    op_ps = scp.tile([128, 512], f32,
                                                     tag="sc", name="sc")
                                    for dc in range(4):
                                        nc.tensor.matmul(
                                            op_ps[:],
                                            at_sb[1][dc][:, t4 * 128:
                                                         (t4 + 1) * 128],
                                            wo[dc][:, et * 512:(et + 1) * 512],
                                            start=(dc == 0), stop=(dc == 3))
                                    nc.scalar.copy(
                                        ob[:, t4, et * 512:(et + 1) * 512],
                                        op_ps[:])
                                    if t4 == 3:
                                        # very last tile: per-half DMAs so the
                                        # final transfer is 128KB and starts
                                        # one ACT-copy earlier
                                        nc.sync.dma_start(
                                            out_d[r0:r0 + 128,
                                                  et * 512:(et + 1) * 512],
                                            ob[:, t4, et * 512:(et + 1) * 512])
                                if t4 < 3:
                                    nc.sync.dma_start(out_d[r0:r0 + 128, :],
                                                      ob[:, t4])

                            # tile 2's o_proj runs as per-t4 filler: between
                            # each diag chunk's scores and its PV (hiding the
                            # mask->exp latency), then overlapping the two
                            # halves' normalize chains
                            ob2 = obp.tile([128, 4, E], bf, tag="ob",
                                           name="ob2")

                            def op2_t4(t4):
                                for et in range(2):
                                    op_ps = scp.tile([128, 512], f32,
                                                     tag="sc", name="sc")
                                    for dc in range(4):
                                        nc.tensor.matmul(
                                            op_ps[:],
                                            at_sb[0][dc][:, t4 * 128:
                                                         (t4 + 1) * 128],
                                            wo[dc][:, et * 512:(et + 1) * 512],
                                            start=(dc == 0), stop=(dc == 3))
                                    nc.scalar.copy(
                                        ob2[:, t4, et * 512:(et + 1) * 512],
                                        op_ps[:])

                            e2 = diag_chunk(2, immediate=True)
                            op2_t4(0)
                            emit_pv(*e2)
                            e3 = diag_chunk(3, immediate=True)
                            op2_t4(1)
                            emit_pv(*e3)
                            rs_ps = app.tile([128, 512], f32, tag="rs",
                                             name="rs")
                            half(0)
                            half(1)
                            op2_t4(2)
                            op2_t4(3)
                            r2 = tok0 + 2 * 512
                            nc.sync.dma_start(
                                out_d[r2:r2 + 512, :].rearrange(
                                    "(f p) e -> p f e", p=128),
                                ob2[:])
                            for t4 in range(4):
                                oproj_t4(t4)
                    if not (b == B - 1):
                        emit_oproj(3)
    nc.compile()
    return nc


def _host_tables():
    inv_freq = 1.0 / (ROPE_BASE ** (np.arange(0, D, 2, dtype=np.float64) / D))
    ang = np.arange(T, dtype=np.float64)[:, None] * inv_freq[None, :]  # [T, D/2]
    import ml_dtypes
    cosdt = np.cos(ang).T.astype(ml_dtypes.bfloat16)                   # [D/2, T]
    sindt = np.sin(ang).T.astype(ml_dtypes.bfloat16)
    csdt = np.ascontiguousarray(np.stack([cosdt, sindt]))  # [2, D/2, T]
    kk = np.arange(128)[:, None]
    qq = np.arange(128)[None, :]
    mtri = np.where(kk <= qq, 0.0, NEG).astype(np.float32)  # [128(k),128(q)]
    return csdt, mtri


def kernel(x, Wq, Wk, Wv, Wo):
    global LAST_RESULTS
    import ml_dtypes
    from concourse import bass_utils

    if "nc" not in _CACHE:
        _CACHE["nc"] = _build()
    nc = _CACHE["nc"]

    bf = ml_dtypes.bfloat16
    x = np.asarray(x, dtype=np.float32)
    Wq = np.asarray(Wq, dtype=np.float32)
    Wk = np.asarray(Wk, dtype=np.float32)
    Wv = np.asarray(Wv, dtype=np.float32)
    Wo = np.asarray(Wo, dtype=np.float32)

    # partition-major swizzles (see kernel docstring): long contiguous DMA rows
    xT = x.reshape(NTOK, E).T.astype(bf)                          # [E, NTOK]
    xsw = np.ascontiguousarray(
        xT.reshape(8, 128, 8, 512).transpose(1, 2, 0, 3))  # [128, g, eo, t]

    def wsw(W, h):  # [E, D] head slice -> [128, eo, d]
        wT = W[h * D:(h + 1) * D, :].T.astype(bf)
        return np.ascontiguousarray(wT.reshape(8, 128, D).transpose(1, 0, 2))

    csdt, mtri = _host_tables()

    in_maps = []
    for h in range(H):
        woT = Wo[:, h * D:(h + 1) * D].T.astype(bf)               # [D, E]
        wvs = wsw(Wv, h)
        boot = np.empty((128, 16, 512), dtype=bf)                 # interleaved
        boot[:, 0::2] = xsw[:, 0]
        boot[:, 1::2] = wvs
        in_maps.append({
            "xsw": xsw,
            "boot": boot,
            "wqsw": wsw(Wq, h),
            "wksw": wsw(Wk, h),
            "wosw": np.ascontiguousarray(
                woT.reshape(4, 128, E).transpose(1, 0, 2)),
            "csdt": csdt,
            "mtri": mtri,
        })

    kwargs = {}
    if PROFILE:
        import sys
        import types
        import trn_agent_boot.trn_boot as _tb
        hook = _tb._ntff_profile_via_ctypes("/opt/axon/libaxon_pjrt.so")
        mod = types.ModuleType("antenv.axon_hooks")
        mod.get_axon_ntff_profile_hook = lambda: hook
        mod.set_axon_ntff_profile_hook = lambda h_: None
        sys.modules["antenv.axon_hooks"] = mod
        bass_utils.upload_artifacts = lambda tmpdir: tmpdir
        kwargs = dict(trace=True, trace_cores=[0])

    res = bass_utils.run_bass_kernel_spmd(
        nc, in_maps, core_ids=list(range(H)), **kwargs)
    LAST_RESULTS = res

    out = res.results[0]["out"].astype(np.float32)
    for h in range(1, H):
        out += res.results[h]["out"].astype(np.float32)
    return out.reshape(B, T, E)



# revision 63
# speedup vs baseline: 1.0102x; 1.0023x over previous
"""Trainium2 Bass kernel: multi-head attention (B=2, T=2048, E=1024, H=8, D=512),
bias-free QKV/O projections + RoPE + causal softmax.

Sharding: head-parallel across 8 NeuronCores. Core h computes head h fully:
  qT/kT = RoPE(Wq_h @ x.T), v = x @ Wv_h.T         (projection phase)
  scoresT[k,q] = kT.T @ qT   (per 512-wide q tile; exact-causal: full-width
    [128,512] chunks below the diagonal block; inside the 512x512 diagonal
    block, chunk j covers only its needed q suffix [128j,512) -- widths
    512/384/256/128 -- with one shared [128,128] triangle mask on each
    chunk's first 128 columns; saves ~16k PE cycles/core vs 256-wide tiles)
  probsT = exp(scale*scoresT + mask)               (no max-subtraction: |s|<=9)
  attnT[d,q] = v.T @ probsT ; rowsum = DVE-accumulated exp partials (bf16)
    reduced across partitions by ONE bf16 ones[128,128] matmul per q tile
    (an f32r stationary would force the 4-byte fp32_mode=HIGH weight path);
    1/rowsum via reciprocal_approx_fast (~5x faster than DVE reciprocal,
    which at ~1.75us/tile sat on the normalize critical path)
  out_h = (attnT/rowsum).T @ Wo_h.T                (partial o_proj, [4096,1024])
Host sums the 8 partial outputs (equivalent to the all-reduce after o_proj).

All matmul operands are bf16 (1 cycle/row at any free size, halved DMA/SBUF);
accumulation stays fp32 in PSUM. cos/sin tables are bf16 (halves their
startup DMA; costs ~3e-4 rel err), resident for both batches; x tiles are
double-buffered with the next tile's DMA issued one tile ahead.
x/w inputs arrive host-swizzled into partition-major [128, ...] layout so
each load is one long contiguous DMA row per partition (~1.7x DMA bw).
Boot blob interleaves x-tile0/wv pairs, loaded as four 512KB quarters into
two tiles (>2 DMA writers per tile races in the dep-tracker), so the first
v passes gate on 512KB instead of 1MB. NW warmup matmuls (which also
produce the rowsum `ones` tile and preload the Exp ACT table) keep the PE
busy and the HAM clock ramped while the first loads stream in.

Output DMAs are merged per q-tile (one [512,1024] transfer) except the very
last tile, which runs per-t4/per-half so the final transfer is small. The
last q tile of batch 1 is processed in two 256-column halves (rowsum MM /
reciprocal / normalize / o_proj per half); tile-2's o_proj t4 sub-blocks
are interleaved as PE filler between each diag chunk's scores and its PV
(hiding mask->exp latency) and across the halves' normalize chains.
Unused DMA queue pools are shrunk to 1 (framework default 3x16 queues).

Measured ~359-360us at 2.4GHz (PE ~94.5% busy, ~345us tensor-active vs
~336us pure-matmul roofline + ~5.5us cold-clock warmup); remaining
overhead is the fixed NEFF preamble (~7us: engine ifetch + 3 barriers)
and epilogue (~7us: ucode zeroes ~100 semaphores serially on 2 engines)
plus ~3.5us final DMA drain.
"""
from contextlib import ExitStack

import numpy as np

B, T, E, H, D = 2, 2048, 1024, 8, 512
NTOK = B * T
SCALE = float(1.0 / np.sqrt(D))
NEG = -1.0e30
ROPE_BASE = 10000.0
NW = 13                   # warmup matmuls (covers startup DMA + HAM ramp)

PROFILE = False          # set True (e.g. from test.py) to trace core 0
LAST_RESULTS = None      # BassKernelResults of the last run when PROFILE

_CACHE = {}


def _build():
    import concourse.tile as tile
    from concourse import bacc, mybir

    f32 = mybir.dt.float32
    f32r = mybir.dt.float32r
    bf = mybir.dt.bfloat16
    AF = mybir.ActivationFunctionType

    nc = bacc.Bacc("TRN2", target_bir_lowering=False, debug=False,
                   enable_asserts=False, num_devices=8)
    # Shrink the DMA queue pools (default 3 pools x 16 queues): the NEFF
    # epilogue zeroes ~2 semaphores per allocated queue one-by-one on the
    # Tensor/Scalar queues (~115ns each, ~7.7us total at 48 queues). This
    # kernel only issues DMAs from the sync (SP) HWDGE queue and never has
    # more than a handful in flight.
    for q in nc.m.queues:
        if q.name != "qSPDynamicHW":
            q.num_queues = 1
    # x/w arrive host-swizzled into partition-major layout so every DMA row
    # is one long contiguous descriptor (8KB) instead of 1KB fragments —
    # ~1.7x higher effective DMA bandwidth on the loads.
    xsw_d = nc.dram_tensor("xsw", [128, 8, 8, 512], bf,
                           kind="ExternalInput").ap()
    # boot blob, interleaved pairs: slot 2e = x-tile0[e], slot 2e+1 = wv[e].
    # Loaded as four 512KB quarters, so the e0-1 accumulation passes of all
    # four v groups of tile 0 can start after just the first quarter lands.
    boot_d = nc.dram_tensor("boot", [128, 16, 512], bf,
                            kind="ExternalInput").ap()
    wq_d = nc.dram_tensor("wqsw", [128, 8, D], bf, kind="ExternalInput").ap()
    wk_d = nc.dram_tensor("wksw", [128, 8, D], bf, kind="ExternalInput").ap()
    wo_d = nc.dram_tensor("wosw", [128, 4, E], bf, kind="ExternalInput").ap()
    # cos+sin packed in bf16 (halved startup DMA): [2(two), D/2, T]
    cs_d = nc.dram_tensor("csdt", [2, D // 2, T], bf, kind="ExternalInput").ap()
    msk_d = nc.dram_tensor("mtri", [128, 128], f32, kind="ExternalInput").ap()
    out_d = nc.dram_tensor("out", [NTOK, E], bf, kind="ExternalOutput").ap()

    cs_r = cs_d.rearrange("two (fo p) t -> p two fo t", p=128)  # [128,2,2,2048]

    with tile.TileContext(nc) as tc, ExitStack() as top:
        wp = top.enter_context(tc.tile_pool(name="wp", bufs=1))
        wq_t = wp.tile([128, 8, D], bf, tag="wq", name="wq")
        wk_t = wp.tile([128, 8, D], bf, tag="wk", name="wk")
        # boot tiles hold wv (resident all kernel) interleaved with x tile 0;
        # two tiles with two DMA writers each (the tile dep-tracker races
        # with more than two DMA writers on one tile)
        bootA = wp.tile([128, 8, 512], bf, tag="bootA", name="bootA")
        bootB = wp.tile([128, 8, 512], bf, tag="bootB", name="bootB")
        bsl = lambda e: (bootA[:, 2 * (e % 4)] if e < 4
                         else bootB[:, 2 * (e % 4)])
        wv = [(bootA[:, 2 * e + 1] if e < 4 else bootB[:, 2 * (e % 4) + 1])
              for e in range(8)]
        x0sl = [bsl(e) for e in range(8)]
        wo_t = wp.tile([128, 4, E], bf, tag="wo", name="wo")
        wo = [wo_t[:, d] for d in range(4)]
        mtri = wp.tile([128, 128], f32, tag="mtri", name="mtri")
        # bf16 stationary for the rowsum matmuls: an f32r stationary forces
        # the fp32_mode=HIGH 4-byte weight-load path (~475ns/MM vs ~215)
        ones = wp.tile([128, 128], bf, tag="ones", name="ones")
        cst = wp.tile([128, 2, 2, T], bf, tag="cst", name="cst")
        csA = cst[:, 0]
        snA = cst[:, 1]
        xb = [wp.tile([128, 8, 512], bf, tag=f"xb{i}", name=f"xb{i}")
              for i in range(2)]

        # ---- warmup: DMA-independent matmuls lift the PE HAM clock gate
        # while the first loads stream in.
        nc.gpsimd.memset(ones[:], 1.0)
        with ExitStack() as wctx:
            warmp = wctx.enter_context(tc.tile_pool(name="warmp", bufs=1))
            wpp = wctx.enter_context(
                tc.tile_pool(name="wpp", bufs=1, space="PSUM"))
            ones0 = warmp.tile([128, 128], bf, tag="ones0", name="ones0")
            nc.vector.memset(ones0[:], 1.0)
            wsrc = warmp.tile([128, 512], bf, tag="wsrc", name="wsrc")
            nc.vector.memset(wsrc[:], 1.0)
            warm_ps = wpp.tile([128, 512], f32, tag="wps", name="wps")
            for w in range(NW):
                nc.tensor.matmul(warm_ps[:], ones0[:], wsrc[:],
                                 start=(w == 0), stop=(w == NW - 1))
            # touch Exp so its ACT table set loads during the DMA-bound
            # startup instead of at the first score tile
            expre = warmp.tile([128, 1], f32, tag="expre", name="expre")
            nc.scalar.activation(expre[:], warm_ps[:, :1], AF.Exp,
                                 scale=0.001)
            nc.vector.tensor_copy(expre[:], expre[:])

        # ---- initial DMAs, need-ordered on the sync HW queue (the other
        # engine queues proved far slower): first v matmuls need xb0+wv,
        # then wq, cos/sin slice 0 (RoPE of tile 0), wk; everything else
        # (x tile 1, the remaining cos/sin, wo, masks) follows.
        nc.sync.dma_start(bootA[:, 0:4], boot_d[:, 0:4])
        nc.sync.dma_start(bootA[:, 4:8], boot_d[:, 4:8])
        nc.sync.dma_start(bootB[:, 0:4], boot_d[:, 8:12])
        nc.sync.dma_start(bootB[:, 4:8], boot_d[:, 12:16])
        nc.sync.dma_start(wq_t[:], wq_d)
        nc.sync.dma_start(cst[:, :, :, 0:512], cs_r[:, :, :, 0:512])
        nc.sync.dma_start(wk_t[:], wk_d)
        nc.sync.dma_start(xb[1][:], xsw_d[:, 1])
        nc.sync.dma_start(cst[:, :, :, 512:T], cs_r[:, :, :, 512:T])
        nc.sync.dma_start(wo_t[:], wo_d)
        nc.sync.dma_start(mtri[:], msk_d)

        for b in range(B):
            tok0 = b * T
            with ExitStack() as bctx:
                qkv = bctx.enter_context(tc.tile_pool(name="qkv", bufs=1))
                # per-token-tile q/k tiles: scores for q-tile n / k-chunk c
                # then depend only on that tile's RoPE writers, not all four
                qTt = [[qkv.tile([128, 512], bf, tag=f"qT{t}_{d}",
                                 name=f"qT{t}_{d}") for d in range(4)]
                       for t in range(4)]
                kTt = [[qkv.tile([128, 512], bf, tag=f"kT{t}_{d}",
                                 name=f"kT{t}_{d}") for d in range(4)]
                       for t in range(4)]
                vv = [qkv.tile([128, D], bf, tag=f"v{t}", name=f"v{t}")
                      for t in range(16)]

                # ----- projection phase: qT/kT (RoPE'd) and v -----
                with ExitStack() as pctx:
                    tp = pctx.enter_context(tc.tile_pool(name="tp", bufs=4))
                    pp = pctx.enter_context(
                        tc.tile_pool(name="pp", bufs=6, space="PSUM"))
                    ppv = pctx.enter_context(
                        tc.tile_pool(name="ppv", bufs=2, space="PSUM"))

                    for tt in range(4):
                        g = 4 * b + tt
                        s0 = tt * 512
                        # g=0 reads x from the boot blob; later tiles from
                        # the ping-pong buffers (xb[0] first used at g=2)
                        xch = (x0sl if g == 0
                               else [xb[g % 2][:, e] for e in range(8)])
                        # prefetch next x tile (one tile ahead; crosses the
                        # batch boundary so b1's first tile lands during
                        # b0's attention phase). g=0's prefetch (tile 1) was
                        # already issued in the startup batch.
                        if 1 <= g < 7:
                            nc.sync.dma_start(xb[(g + 1) % 2][:],
                                              xsw_d[:, g + 1])

                        def emit_v():
                            for t4 in range(4):
                                ps_t = ppv.tile([128, 512], f32, tag="ppv",
                                                name="ppv")
                                for e in range(8):
                                    nc.tensor.matmul(
                                        ps_t[:],
                                        xch[e][:, t4 * 128:(t4 + 1) * 128],
                                        wv[e][:],
                                        start=(e == 0), stop=(e == 7))
                                nc.scalar.copy(vv[tt * 4 + t4][:], ps_t[:])

                        def emit_v_split():
                            # tile 0 of batch 0: each e-pair's accumulation
                            # passes gate only on its own boot quarter, so
                            # real PE work starts after 512KB of DMA
                            vps = [(ppv if t4 < 2 else pp).tile(
                                [128, 512], f32,
                                tag=("ppv" if t4 < 2 else "pp"),
                                name="vps") for t4 in range(4)]
                            for lo, hi in ((0, 2), (2, 4), (4, 6), (6, 8)):
                                for t4 in range(4):
                                    for e in range(lo, hi):
                                        nc.tensor.matmul(
                                            vps[t4][:],
                                            xch[e][:, t4 * 128:(t4 + 1) * 128],
                                            wv[e][:],
                                            start=(e == 0), stop=(e == 7))
                            for t4 in range(4):
                                nc.scalar.copy(vv[t4][:], vps[t4][:])
                        # v first (its ACT-copy evacuation has no cos/sin
                        # dependency) except on the last token tile, where
                        # qk-first lets the P phase end with a short ACT tail
                        # instead of a long RoPE DVE tail.
                        if tt < 3:
                            if g == 0:
                                emit_v_split()
                            else:
                                emit_v()
                        for w_t, dstT in ((wq_t, qTt[tt]), (wk_t, kTt[tt])):
                            for i, j, fo in ((0, 2, 0), (1, 3, 1)):
                                ps2 = []
                                for dc in (i, j):
                                    ps_t = pp.tile([128, 512], f32, tag="pp",
                                                   name="pp")
                                    for e in range(8):
                                        nc.tensor.matmul(
                                            ps_t[:],
                                            w_t[:, e, dc * 128:(dc + 1) * 128],
                                            xch[e],
                                            start=(e == 0), stop=(e == 7))
                                    ps2.append(ps_t)
                                pi, pj = ps2
                                c_ = csA[:, fo, s0:s0 + 512]
                                s_ = snA[:, fo, s0:s0 + 512]
                                t0 = tp.tile([128, 512], f32, tag="rt", name="rt")
                                t1 = tp.tile([128, 512], f32, tag="rt", name="rt")
                                nc.vector.tensor_mul(t0[:], pi[:], c_)
                                nc.vector.tensor_mul(t1[:], pj[:], s_)
                                nc.vector.tensor_sub(dstT[i][:], t0[:], t1[:])
                                t2 = tp.tile([128, 512], f32, tag="rt", name="rt")
                                t3 = tp.tile([128, 512], f32, tag="rt", name="rt")
                                nc.vector.tensor_mul(t2[:], pi[:], s_)
                                nc.vector.tensor_mul(t3[:], pj[:], c_)
                                nc.vector.tensor_add(dstT[j][:], t2[:], t3[:])
                        if tt == 3:
                            emit_v()

                # ----- attention + o_proj phase -----
                with ExitStack() as actx:
                    ep = actx.enter_context(tc.tile_pool(name="ep", bufs=6))
                    atp = actx.enter_context(tc.tile_pool(name="atp", bufs=1))
                    ivp = actx.enter_context(tc.tile_pool(name="ivp", bufs=2))
                    rsb = actx.enter_context(tc.tile_pool(name="rsb", bufs=2))
                    obp = actx.enter_context(tc.tile_pool(name="obp", bufs=2))
                    scp = actx.enter_context(
                        tc.tile_pool(name="scp", bufs=3, space="PSUM"))
                    app = actx.enter_context(
                        tc.tile_pool(name="app", bufs=1, space="PSUM"))

                    def emit_oproj(n, split=False):
                        # split=True (very last tile): per-t4 output DMAs so
                        # the final transfer is small and starts right after
                        # its ACT-copy instead of waiting for all four.
                        q0 = n * 512
                        ob = obp.tile([128, 4, E], bf, tag="ob", name="ob")
                        for t4 in range(4):
                            for et in range(2):
                                op_ps = scp.tile([128, 512], f32, tag="sc",
                                                 name="sc")
                                for dc in range(4):
                                    nc.tensor.matmul(
                                        op_ps[:],
                                        at_sb[n % 2][dc][:, t4 * 128:(t4 + 1) * 128],
                                        wo[dc][:, et * 512:(et + 1) * 512],
                                        start=(dc == 0), stop=(dc == 3))
                                nc.scalar.copy(
                                    ob[:, t4, et * 512:(et + 1) * 512], op_ps[:])
                            if split:
                                r0 = tok0 + q0 + t4 * 128
                                nc.sync.dma_start(out_d[r0:r0 + 128, :],
                                                  ob[:, t4])
                        if not split:
                            r0 = tok0 + q0
                            nc.sync.dma_start(
                                out_d[r0:r0 + 512, :].rearrange(
                                    "(f p) e -> p f e", p=128),
                                ob[:])

                    at_sb = {0: None, 1: None}
                    for n in range(4):
                        q0 = n * 512
                        noff = 4 * n          # full-width k chunks below diag
                        attn_ps = [app.tile([128, 512], f32, tag=f"attn{d}",
                                            name=f"attn{d}") for d in range(4)]

                        rs_acc = rsb.tile([128, 512], bf, tag="rsa",
                                          name="rsa")

                        # rowsum partial accumulation (DVE); the first two
                        # full-width exp tiles are paired into one 2-input
                        # add; diag chunks add into suffix ranges [qoff:512]
                        rst = {"f": None, "fi": False}

                        def rs_add(ex, qoff, w, rst=rst, rs_acc=rs_acc):
                            if not rst["fi"]:
                                if w == 512:
                                    if rst["f"] is None:
                                        rst["f"] = ex
                                        return
                                    nc.vector.tensor_add(
                                        rs_acc[:], rst["f"][:], ex[:])
                                    rst["fi"] = True
                                    return
                                # lone 512-wide first touch (n=0 diag j=0):
                                # materialize it, then add this suffix
                                nc.vector.tensor_copy(rs_acc[:], rst["f"][:])
                                rst["fi"] = True
                            dst = rs_acc[:, qoff:qoff + w]
                            nc.vector.tensor_add(dst, dst, ex[:])

                        def emit_pv(ex, c, qoff, w, st, sp,
                                    attn_ps=attn_ps):
                            for dc in range(4):
                                nc.tensor.matmul(
                                    attn_ps[dc][:, qoff:qoff + w],
                                    vv[c][:, dc * 128:(dc + 1) * 128], ex[:],
                                    start=st, stop=sp)

                        pending = []

                        def push(entry):
                            # rowsum add at push time (it only needs the exp
                            # tile, ready now) — keeps the DVE chain tracking
                            # the exp stream so the q-tile's trailing
                            # rs->recip->normalize path isn't serialized
                            # behind a flush backlog
                            rs_add(entry[0], entry[2], entry[3])
                            pending.append(entry)
                            if len(pending) > 4:
                                emit_pv(*pending.pop(0))

                        # full-width chunks strictly below the diagonal block
                        for c in range(noff):
                            sc_ps = scp.tile([128, 512], f32, tag="sc",
                                             name="sc")
                            for dc in range(4):
                                nc.tensor.matmul(
                                    sc_ps[:],
                                    kTt[c // 4][dc][:, (c % 4) * 128:
                                                    (c % 4 + 1) * 128],
                                    qTt[n][dc][:],
                                    start=(dc == 0), stop=(dc == 3))
                            ex = ep.tile([128, 512], bf, tag="ex", name="ex")
                            nc.scalar.activation(ex[:], sc_ps[:], AF.Exp,
                                                 scale=SCALE)
                            push((ex, c, 0, 512, c == 0, False))
                        # diagonal 512x512 block: chunk j (k in [128j,128j+128)
                        # within the block) covers only its causally-needed q
                        # suffix [128j, 512); the first 128 q columns of each
                        # chunk sit on the diagonal and get the triangle mask
                        def diag_chunk(j, immediate=False):
                            c = noff + j
                            qoff = j * 128
                            w = 512 - qoff
                            sc_ps = scp.tile([128, 512], f32, tag="sc",
                                             name="sc")
                            for dc in range(4):
                                nc.tensor.matmul(
                                    sc_ps[:, 0:w],
                                    kTt[c // 4][dc][:, (c % 4) * 128:
                                                    (c % 4 + 1) * 128],
                                    qTt[n][dc][:, qoff:512],
                                    start=(dc == 0), stop=(dc == 3))
                            nc.vector.tensor_add(
                                sc_ps[:, 0:128], sc_ps[:, 0:128], mtri[:])
                            ex = ep.tile([128, w], bf,
                                         tag=("ex" if j == 0 else f"exd{j}"),
                                         name="exd")
                            nc.scalar.activation(ex[:], sc_ps[:, 0:w],
                                                 AF.Exp, scale=SCALE)
                            entry = (ex, c, qoff, w,
                                     noff == 0 and j == 0, j == 3)
                            if immediate:
                                rs_add(ex, qoff, w)
                                return entry
                            push(entry)

                        def normalize(dst, lo, w, inv):
                            for dc in range(4):
                                nc.vector.tensor_mul(dst[dc][:, lo:lo + w],
                                                     attn_ps[dc][:, lo:lo + w],
                                                     inv[:])

                        last = (b == B - 1 and n == 3)
                        if not last:
                            for j in range(4):
                                diag_chunk(j)
                            for entry in pending:
                                emit_pv(*entry)
                            # cross-partition reduce of the exp partials: ONE
                            # ones-matmul per q tile (broadcasts rowsum to all
                            # partitions), then normalize + evacuate
                            rs_ps = app.tile([128, 512], f32, tag="rs",
                                             name="rs")
                            nc.tensor.matmul(rs_ps[:], ones[:], rs_acc[:],
                                             start=True, stop=True)
                            inv = ivp.tile([128, 512], f32, tag="inv",
                                           name="inv")
                            nc.vector.reciprocal_approx_fast(
                                out=inv[:], in_=rs_ps[:])
                            at_sb[n % 2] = [
                                atp.tile([128, 512], bf, tag=f"at{n % 2}_{dc}",
                                         name=f"at{n % 2}_{dc}")
                                for dc in range(4)]
                            normalize(at_sb[n % 2], 0, 512, inv)
                            if n > 0:
                                emit_oproj(n - 1)
                        else:
                            # last q tile of the last batch: process in two
                            # 256-wide halves so rowsum/normalize/o_proj of
                            # the left half overlap the right half's PV, and
                            # the end-of-kernel serial tail shrinks
                            diag_chunk(0)
                            diag_chunk(1)
                            for entry in pending:
                                emit_pv(*entry)
                            at_sb[1] = [
                                atp.tile([128, 512], bf, tag=f"at1_{dc}",
                                         name=f"at1_{dc}")
                                for dc in range(4)]
                            ob = obp.tile([128, 4, E], bf, tag="ob",
                                          name="ob")

                            def half(h):
                                lo = h * 256
                                nc.tensor.matmul(rs_ps[:, lo:lo + 256],
                                                 ones[:],
                                                 rs_acc[:, lo:lo + 256],
                                                 start=True, stop=True)
                                inv = ivp.tile([128, 256], f32,
                                               tag=f"invh{h}", name="invh")
                                nc.vector.reciprocal_approx_fast(
                                    out=inv[:], in_=rs_ps[:, lo:lo + 256])
                                normalize(at_sb[1], lo, 256, inv)

                            def oproj_t4(t4):
                                r0 = tok0 + q0 + t4 * 128
                                for et in range(2):
                                    op_ps = scp.tile([128, 512], f32,
                                                     tag="sc", name="sc")
                                    for dc in range(4):
                                        nc.tensor.matmul(
                                            op_ps[:],
                                            at_sb[1][dc][:, t4 * 128:
                                                         (t4 + 1) * 128],
                                            wo[dc][:, et * 512:(et + 1) * 512],
                                            start=(dc == 0), stop=(dc == 3))
                                    nc.scalar.copy(
                                        ob[:, t4, et * 512:(et + 1) * 512],
                                        op_ps[:])
                                    if t4 == 3:
                                        # very last tile: per-half DMAs so the
                                        # final transfer is 128KB and starts
                                        # one ACT-copy earlier
                                        nc.sync.dma_start(
                                            out_d[r0:r0 + 128,
                                                  et * 512:(et + 1) * 512],
                                            ob[:, t4, et * 512:(et + 1) * 512])
                                if t4 < 3:
                                    nc.sync.dma_start(out_d[r0:r0 + 128, :],
                                                      ob[:, t4])

                            # tile 2's o_proj runs as per-t4 filler: between
                            # each diag chunk's scores and its PV (hiding the
                            # mask->exp latency), then overlapping the two
                            # halves' normalize chains
                            ob2 = obp.tile([128, 4, E], bf, tag="ob",
                                           name="ob2")

                            def op2_t4(t4):
                                for et in range(2):
                                    op_ps = scp.tile([128, 512], f32,
                                                     tag="sc", name="sc")
                                    for dc in range(4):
                                        nc.tensor.matmul(
                                            op_ps[:],
                                            at_sb[0][dc][:, t4 * 128:
                                                         (t4 + 1) * 128],
                                            wo[dc][:, et * 512:(et + 1) * 512],
                                            start=(dc == 0), stop=(dc == 3))
                                    nc.scalar.copy(
                                        ob2[:, t4, et * 512:(et + 1) * 512],
                                        op_ps[:])

                            e2 = diag_chunk(2, immediate=True)
                            op2_t4(0)
                            emit_pv(*e2)
                            e3 = diag_chunk(3, immediate=True)
                            op2_t4(1)
                            emit_pv(*e3)
                            rs_ps = app.tile([128, 512], f32, tag="rs",
                                             name="rs")
                            half(0)
                            half(1)
                            op2_t4(2)
                            op2_t4(3)
                            r2 = tok0 + 2 * 512
                            nc.sync.dma_start(
                                out_d[r2:r2 + 512, :].rearrange(
                                    "(f p) e -> p f e", p=128),
                                ob2[:])
                            for t4 in range(4):
                                oproj_t4(t4)
                    if not (b == B - 1):
                        emit_oproj(3)
    nc.compile()
    return nc


def _host_tables():
    inv_freq = 1.0 / (ROPE_BASE ** (np.arange(0, D, 2, dtype=np.float64) / D))
    ang = np.arange(T, dtype=np.float64)[:, None] * inv_freq[None, :]  # [T, D/2]
    import ml_dtypes
    cosdt = np.cos(ang).T.astype(ml_dtypes.bfloat16)                   # [D/2, T]
    sindt = np.sin(ang).T.astype(ml_dtypes.bfloat16)
    csdt = np.ascontiguousarray(np.stack([cosdt, sindt]))  # [2, D/2, T]
    kk = np.arange(128)[:, None]
    qq = np.arange(128)[None, :]
    mtri = np.where(kk <= qq, 0.0, NEG).astype(np.float32)  # [128(k),128(q)]
    return csdt, mtri


def kernel(x, Wq, Wk, Wv, Wo):
    global LAST_RESULTS
    import ml_dtypes
    from concourse import bass_utils

    if "nc" not in _CACHE:
        _CACHE["nc"] = _build()
    nc = _CACHE["nc"]

    bf = ml_dtypes.bfloat16
    x = np.asarray(x, dtype=np.float32)
    Wq = np.asarray(Wq, dtype=np.float32)
    Wk = np.asarray(Wk, dtype=np.float32)
    Wv = np.asarray(Wv, dtype=np.float32)
    Wo = np.asarray(Wo, dtype=np.float32)

    # partition-major swizzles (see kernel docstring): long contiguous DMA rows
    xT = x.reshape(NTOK, E).T.astype(bf)                          # [E, NTOK]
    xsw = np.ascontiguousarray(
        xT.reshape(8, 128, 8, 512).transpose(1, 2, 0, 3))  # [128, g, eo, t]

    def wsw(W, h):  # [E, D] head slice -> [128, eo, d]
        wT = W[h * D:(h + 1) * D, :].T.astype(bf)
        return np.ascontiguousarray(wT.reshape(8, 128, D).transpose(1, 0, 2))

    csdt, mtri = _host_tables()

    in_maps = []
    for h in range(H):
        woT = Wo[:, h * D:(h + 1) * D].T.astype(bf)               # [D, E]
        wvs = wsw(Wv, h)
        boot = np.empty((128, 16, 512), dtype=bf)                 # interleaved
        boot[:, 0::2] = xsw[:, 0]
        boot[:, 1::2] = wvs
        in_maps.append({
            "xsw": xsw,
            "boot": boot,
            "wqsw": wsw(Wq, h),
            "wksw": wsw(Wk, h),
            "wosw": np.ascontiguousarray(
                woT.reshape(4, 128, E).transpose(1, 0, 2)),
            "csdt": csdt,
            "mtri": mtri,
        })

    kwargs = {}
    if PROFILE:
        import sys
        import types
        import trn_agent_boot.trn_boot as _tb
        hook = _tb._ntff_profile_via_ctypes("/opt/axon/libaxon_pjrt.so")
        mod = types.ModuleType("antenv.axon_hooks")
        mod.get_axon_ntff_profile_hook = lambda: hook
        mod.set_axon_ntff_profile_hook = lambda h_: None
        sys.modules["antenv.axon_hooks"] = mod
        bass_utils.upload_artifacts = lambda tmpdir: tmpdir
        kwargs = dict(trace=True, trace_cores=[0])

    res = bass_utils.run_bass_kernel_spmd(
        nc, in_maps, core_ids=list(range(H)), **kwargs)
    LAST_RESULTS = res

    out = res.results[0]["out"].astype(np.float32)
    for h in range(1, H):
        out += res.results[h]["out"].astype(np.float32)
    return out.reshape(B, T, E)

